# revision 4
# baseline (speedup 1.0000x reference)
"""ContextualActorCritic Trainium2 kernel.

Strategy (moe_routing): host sorts each core's 8192 rows by leaf id and pads
each leaf group to a multiple of 128 rows, so every 128-row device tile has a
single leaf head. Per-core device program (identical SPMD program, per-core
data):
  phase 2 (per 512-batch iter): L1 matmuls (bias folded via augmented ones
  row, K=65) -> one tanh [128,2048]; L2 matmuls + K=1 bias matmuls -> one
  tanh [128,2048]; V3 value matmul; per-row-tile head matmuls (leaf-gathered
  Wh passed as data); z copied PSUM->SBUF with masked bias added (DVE).
  phase 4: big-tile softmax over [128, 32*T] (exp / segmented reduces /
  one-hot action extraction / entropy).
Matmuls run in float32r (full PE rate at N>=512; ~2e-4 rel err).
Outputs unsorted on host.
"""

import numpy as np

B = 65536
OBS = 64
H = 256
L = 16
A = 32
NCORES = 8
BC = B // NCORES  # 8192 rows per core

_CACHE = {}


# ---------------------------------------------------------------------------
# walrus in this container allows only ONE sync-wait per instruction: hoist
# extra waits onto injected NoOps placed just before the instruction.
def _split_multi_waits(nc):
    import concourse.mybir as mybir

    ctr = 0
    for f in nc.m.functions:
        for blk in f.blocks:
            insts = list(blk.instructions)
            out = []
            changed = False
            for inst in insts:
                si = inst.sync_info
                waits = list(si.on_wait) if (si is not None and si.on_wait) else []
                if len(waits) > 1:
                    changed = True
                    for w in waits[:-1]:
                        ctr += 1
                        nop = mybir.InstNoOp(name=f"SWT-{ctr}", ins=[], outs=[])
                        nop.engine = inst.engine
                        nop.sync_info = mybir.SyncInfo(on_wait=[w], on_update=[])
                        out.append(nop)
                    inst.sync_info = mybir.SyncInfo(
                        on_wait=[waits[-1]], on_update=list(si.on_update or [])
                    )
                out.append(inst)
            if changed:
                blk.instructions.clear()
                blk.instructions.extend(out)


def _build_program(T):
    """Build the Bass program for T row-tiles (128 rows each) per core."""
    import concourse.bass as bass
    import concourse.mybir as mybir
    import concourse.tile as tile

    F32 = mybir.dt.float32
    F32R = mybir.dt.float32r
    AF = mybir.ActivationFunctionType
    ALU = mybir.AluOpType
    AX = mybir.AxisListType

    BP = 128 * T          # padded rows per core
    NIT = BP // 512       # 512-batch iterations
    ZW = 32 * T           # z matrix width

    nc = bass.Bass("TRN2")
    # inputs
    d_obsT = nc.dram_tensor("obsT", [OBS + 1, BP], F32, kind="ExternalInput")
    d_w1v1 = nc.dram_tensor("w1v1", [OBS + 1, 2 * H], F32, kind="ExternalInput")
    d_w2v2 = nc.dram_tensor("w2v2", [128, 8 * 128], F32, kind="ExternalInput")
    d_bias2 = nc.dram_tensor("bias2", [1, 512], F32, kind="ExternalInput")
    d_v3 = nc.dram_tensor("v3", [128, 2], F32, kind="ExternalInput")
    d_bv3 = nc.dram_tensor("bv3", [1, 1], F32, kind="ExternalInput")
    d_whT = nc.dram_tensor("whT", [128, 2 * ZW], F32, kind="ExternalInput")
    d_mbias = nc.dram_tensor("mbias", [128, ZW], F32, kind="ExternalInput")
    d_onehot = nc.dram_tensor("onehot", [128, ZW], F32, kind="ExternalInput")
    d_out = nc.dram_tensor("out", [3, BP], F32, kind="ExternalOutput")

    with tile.TileContext(nc) as tc:
        with (
            tc.tile_pool(name="const", bufs=1) as cpool,
            tc.tile_pool(name="big", bufs=1) as bpool,
            tc.tile_pool(name="act", bufs=2) as apool,
            tc.tile_pool(name="soft", bufs=1) as spool,
            tc.tile_pool(name="ps", bufs=1, space="PSUM") as pp,
        ):
            # ---- constants / weights ----
            w1v1 = cpool.tile([OBS + 1, 2 * H], F32R)
            nc.gpsimd.dma_start(w1v1[:], d_w1v1[:])
            w2v2 = cpool.tile([128, 8 * 128], F32R)
            nc.gpsimd.dma_start(w2v2[:], d_w2v2[:])
            bias2 = cpool.tile([1, 512], F32R)
            nc.gpsimd.dma_start(bias2[:], d_bias2[:])
            v3 = cpool.tile([128, 2], F32R)
            nc.gpsimd.dma_start(v3[:], d_v3[:])
            bv3 = cpool.tile([1, 1], F32)
            nc.gpsimd.dma_start(bv3[:], d_bv3[:])
            ones_f = cpool.tile([1, 512], F32)
            nc.vector.memset(ones_f[:], 1.0)
            ones = ones_f.bitcast(F32R)

            whT = bpool.tile([128, 2 * ZW], F32R)
            nc.gpsimd.dma_start(whT[:], d_whT[:])
            mbias = bpool.tile([128, ZW], F32)
            nc.gpsimd.dma_start(mbias[:], d_mbias[:])
            onehot = bpool.tile([128, ZW], F32)
            nc.gpsimd.dma_start(onehot[:], d_onehot[:])

            obsT = bpool.tile([OBS + 1, BP], F32R)
            for i in range(NIT):
                nc.gpsimd.dma_start(
                    obsT[:, i * 512 : (i + 1) * 512],
                    d_obsT[:, i * 512 : (i + 1) * 512],
                )

            z_sb = spool.tile([128, ZW], F32)
            val_sb = spool.tile([1, BP], F32)
            lp_sb = spool.tile([128, T], F32)
            ent_sb = spool.tile([128, T], F32)

            # ---- phase 2: backbone + value + heads, per 512-batch iter ----
            for i in range(NIT):
                bs = i * 512
                rhs_obs = obsT[:, bs : bs + 512]
                pL1 = pp.tile([128, 2048], F32, tag="A", name="pL1")
                # L1: [h1c0 | h1c1 | hv1c0 | hv1c1], bias via ones row (K=65)
                for j in range(4):
                    nc.tensor.matmul(
                        pL1[:, j * 512 : (j + 1) * 512],
                        w1v1[:, j * 128 : (j + 1) * 128],
                        rhs_obs,
                        start=True,
                        stop=True,
                    )
                sb1 = apool.tile([128, 2048], F32R, tag="sb1")
                nc.scalar.activation(sb1[:], pL1[:], AF.Tanh)

                pL2 = pp.tile([128, 2048], F32, tag="B", name="pL2")
                # L2: [hc0 | hc1 | hvc0 | hvc1]
                # w2v2 layout col j*128: j = net*4 + kc*2 + mc
                for net in range(2):  # 0: policy (W2), 1: value (V2)
                    for mc in range(2):
                        dst = pL2[:, (net * 2 + mc) * 512 : (net * 2 + mc + 1) * 512]
                        for kc in range(2):
                            jw = net * 4 + kc * 2 + mc
                            nc.tensor.matmul(
                                dst,
                                w2v2[:, jw * 128 : (jw + 1) * 128],
                                sb1[:, (net * 2 + kc) * 512 : (net * 2 + kc + 1) * 512],
                                start=(kc == 0),
                                stop=False,
                            )
                        # K=1 bias matmul: bias2 col block (net*2+mc)
                        nc.tensor.matmul(
                            dst,
                            bias2[:, (net * 2 + mc) * 128 : (net * 2 + mc + 1) * 128],
                            ones[:, 0:512],
                            start=False,
                            stop=True,
                        )
                sb2 = apool.tile([128, 2048], F32R, tag="sb2")
                nc.scalar.activation(sb2[:], pL2[:], AF.Tanh)

                # aux psum reuses L2 slot (tag B): z tiles + value row
                aux = pp.tile([128, 2048], F32, tag="B", name="aux")
                # value: out [1, 512] at aux[0:1, 512:1024]
                vdst = aux[0:1, 512:1024]
                nc.tensor.matmul(vdst, v3[:, 0:1], sb2[:, 1024:1536], start=True, stop=False)
                nc.tensor.matmul(vdst, v3[:, 1:2], sb2[:, 1536:2048], start=False, stop=True)
                nc.vector.tensor_scalar(
                    val_sb[0:1, bs : bs + 512], vdst, bv3[0:1, 0:1], None, ALU.add
                )
                # heads: 4 row-tiles of 128
                for j in range(4):
                    t = 4 * i + j
                    zdst = aux[:, 32 * j : 32 * j + 32]
                    nc.tensor.matmul(
                        zdst,
                        sb2[:, j * 128 : (j + 1) * 128],
                        whT[:, 32 * t : 32 * t + 32],
                        start=True,
                        stop=False,
                    )
                    nc.tensor.matmul(
                        zdst,
                        sb2[:, 512 + j * 128 : 512 + (j + 1) * 128],
                        whT[:, ZW + 32 * t : ZW + 32 * t + 32],
                        start=False,
                        stop=True,
                    )
                    # z_sb = z + mbias (masked bias; -1e9 on invalid actions)
                    nc.vector.scalar_tensor_tensor(
                        z_sb[:, 32 * t : 32 * t + 32],
                        zdst,
                        0.0,
                        mbias[:, 32 * t : 32 * t + 32],
                        ALU.add,
                        ALU.add,
                    )

            # ---- phase 4: softmax / logprob / entropy over [128, ZW] ----
            e_sb = spool.tile([128, ZW], F32)
            nc.scalar.activation(e_sb[:], z_sb[:], AF.Exp)
            s = spool.tile([128, T], F32)
            nc.vector.tensor_reduce(
                s[:], e_sb.rearrange("p (t a) -> p t a", a=A), AX.X, ALU.add
            )
            logZ = spool.tile([128, T], F32)
            nc.scalar.activation(logZ[:], s[:], AF.Ln)
            # z_sel = sum(z * onehot) per 32-block
            zoh = spool.tile([128, ZW], F32)
            nc.vector.tensor_mul(zoh[:], z_sb[:], onehot[:])
            zsel = spool.tile([128, T], F32)
            nc.vector.tensor_reduce(
                zsel[:], zoh.rearrange("p (t a) -> p t a", a=A), AX.X, ALU.add
            )
            # S1 = sum(e * z) per 32-block (e==0 kills masked -1e9 cols)
            ez = spool.tile([128, ZW], F32)
            nc.vector.tensor_mul(ez[:], e_sb[:], z_sb[:])
            s1 = spool.tile([128, T], F32)
            nc.vector.tensor_reduce(
                s1[:], ez.rearrange("p (t a) -> p t a", a=A), AX.X, ALU.add
            )
            # lp = zsel - logZ ; ent = logZ - s1/s
            nc.vector.tensor_sub(lp_sb[:], zsel[:], logZ[:])
            rcp = spool.tile([128, T], F32)
            nc.vector.reciprocal(rcp[:], s[:])
            t1 = spool.tile([128, T], F32)
            nc.vector.tensor_mul(t1[:], s1[:], rcp[:])
            nc.vector.tensor_sub(ent_sb[:], logZ[:], t1[:])

            # ---- outputs ----
            nc.gpsimd.dma_start(d_out[0].rearrange("(p t) -> p t", p=128), lp_sb[:])
            nc.gpsimd.dma_start(d_out[1].rearrange("(p t) -> p t", p=128), ent_sb[:])
            nc.gpsimd.dma_start(d_out[2:3, :], val_sb[:])

    _split_multi_waits(nc)
    return nc


def _prep_host(obs, actions, leaf_ids, counts, Wh, bh):
    """Sort+pad rows per core; build per-core device inputs."""
    actions = np.asarray(actions).astype(np.int64)
    leaf_ids = np.asarray(leaf_ids).astype(np.int64)
    counts = np.asarray(counts).astype(np.int64)
    obs = np.asarray(obs, dtype=np.float32)

    # per-core padded tile counts -> global T
    per_core = []
    T = 0
    for c in range(NCORES):
        lid = leaf_ids[c * BC : (c + 1) * BC]
        order = np.argsort(lid, kind="stable")
        cnt = np.bincount(lid, minlength=L)
        tiles = int(np.sum((cnt + 127) // 128))
        T = max(T, tiles)
        per_core.append((order, cnt))
    T = (T + 3) // 4 * 4  # batch iterations cover 4 tiles (512 rows) each
    # src_idx / tile_leaf per core with shared T
    cores = []
    for c in range(NCORES):
        order, cnt = per_core[c]
        src_idx = np.full(128 * T, -1, dtype=np.int64)
        tile_leaf = np.full(T, L - 1, dtype=np.int64)
        pos = 0
        t = 0
        off = 0
        for l in range(L):
            n = int(cnt[l])
            if n == 0:
                continue
            src_idx[pos : pos + n] = order[off : off + n]
            off += n
            ntile = (n + 127) // 128
            tile_leaf[t : t + ntile] = l
            t += ntile
            pos = t * 128
        cores.append((src_idx, tile_leaf))
    return obs, actions, leaf_ids, counts, T, cores


def kernel(**inputs):
    from concourse.bass_utils import run_bass_kernel_spmd

    obs = inputs["obs"]
    actions = inputs["actions"]
    leaf_ids = inputs["leaf_ids"]
    counts = inputs["counts"]
    W1 = np.asarray(inputs["W1"], np.float32)
    b1 = np.asarray(inputs["b1"], np.float32)
    W2 = np.asarray(inputs["W2"], np.float32)
    b2 = np.asarray(inputs["b2"], np.float32)
    Wh = np.asarray(inputs["Wh"], np.float32)
    bh = np.asarray(inputs["bh"], np.float32)
    V1 = np.asarray(inputs["V1"], np.float32)
    bv1 = np.asarray(inputs["bv1"], np.float32)
    V2 = np.asarray(inputs["V2"], np.float32)
    bv2 = np.asarray(inputs["bv2"], np.float32)
    V3 = np.asarray(inputs["V3"], np.float32)
    bv3 = np.asarray(inputs["bv3"], np.float32)

    obs, actions, leaf_ids, counts, T, cores = _prep_host(
        obs, actions, leaf_ids, counts, Wh, bh
    )
    BP = 128 * T
    ZW = 32 * T

    if T not in _CACHE:
        _CACHE[T] = _build_program(T)
    nc = _CACHE[T]

    # shared weight tensors
    w1v1 = np.zeros((OBS + 1, 2 * H), np.float32)
    w1v1[:OBS, :H] = W1
    w1v1[OBS, :H] = b1
    w1v1[:OBS, H:] = V1
    w1v1[OBS, H:] = bv1
    w2v2 = np.zeros((128, 8 * 128), np.float32)
    for net, (Wm, _) in enumerate(((W2, b2), (V2, bv2))):
        Wm = [W2, V2][net]
        for kc in range(2):
            for mc in range(2):
                jw = net * 4 + kc * 2 + mc
                w2v2[:, jw * 128 : (jw + 1) * 128] = Wm[
                    kc * 128 : (kc + 1) * 128, mc * 128 : (mc + 1) * 128
                ]
    bias2 = np.zeros((1, 512), np.float32)
    bias2[0, 0:128] = b2[0:128]
    bias2[0, 128:256] = b2[128:256]
    bias2[0, 256:384] = bv2[0:128]
    bias2[0, 384:512] = bv2[128:256]
    v3t = np.zeros((128, 2), np.float32)
    v3t[:, 0] = V3[0:128, 0]
    v3t[:, 1] = V3[128:256, 0]
    bv3t = np.array([[float(bv3[0])]], np.float32)

    # masked head bias per leaf: [L, A]
    mb = np.where(np.arange(A)[None, :] < counts[:, None], bh, -1e9).astype(np.float32)
    whT_l = Wh.transpose(0, 2, 1).astype(np.float32)  # [L, H, A]

    in_maps = []
    for c in range(NCORES):
        src_idx, tile_leaf = cores[c]
        valid = src_idx >= 0
        rows = np.zeros((BP, OBS), np.float32)
        rows[valid] = obs[c * BC : (c + 1) * BC][src_idx[valid]]
        obsT = np.ones((OBS + 1, BP), np.float32)
        obsT[:OBS] = rows.T

        whT_sel = np.zeros((128, 2 * ZW), np.float32)
        mbias_sel = np.zeros((128, ZW), np.float32)
        for t in range(T):
            l = int(tile_leaf[t])
            whT_sel[:, 32 * t : 32 * t + 32] = whT_l[l, 0:128, :]
            whT_sel[:, ZW + 32 * t : ZW + 32 * t + 32] = whT_l[l, 128:256, :]
            mbias_sel[:, 32 * t : 32 * t + 32] = mb[l][None, :]

        onehot = np.zeros((128, ZW), np.float32)
        j = np.nonzero(valid)[0]
        src = src_idx[j]
        lid = leaf_ids[c * BC : (c + 1) * BC][src]
        acl = np.clip(actions[c * BC : (c + 1) * BC][src], 0, counts[lid] - 1)
        onehot[j % 128, 32 * (j // 128) + acl] = 1.0

        in_maps.append(
            dict(
                obsT=obsT,
                w1v1=w1v1,
                w2v2=w2v2,
                bias2=bias2,
                v3=v3t,
                bv3=bv3t,
                whT=whT_sel,
                mbias=mbias_sel,
                onehot=onehot,
            )
        )

    res = run_bass_kernel_spmd(nc, in_maps, core_ids=list(range(NCORES)))

    log_probs = np.zeros(B, np.float32)
    entropies = np.zeros(B, np.float32)
    values = np.zeros(B, np.float32)
    for c in range(NCORES):
        out = res.results[c]["out"]
        src_idx, _ = cores[c]
        valid = src_idx >= 0
        j = np.nonzero(valid)[0]
        dst = c * BC + src_idx[j]
        lp_pad = out[0].reshape(128, T).T.ravel()
        ent_pad = out[1].reshape(128, T).T.ravel()
        log_probs[dst] = lp_pad[j]
        entropies[dst] = ent_pad[j]
        values[dst] = out[2][j]
    return log_probs, entropies, values
